# revision 2
# baseline (speedup 1.0000x reference)
"""AdaPT int8-quantized Linear on 8 TRN2 NeuronCores.

out = round_int8(x * 127/amax(x)) @ round_int8(w * 127/amax(w)).T * dequant + bias

Exactness: int8 values (|v| <= 127) are exact in bf16; their products
(<= 16129) and the accumulated partial sums (~1e5 << 2^24) are exact in
fp32 PSUM, so a bf16 TensorE matmul reproduces the int8 x int8 -> int32
matmul bit-exactly at full bf16 throughput.

Sharding: x row-parallel (1024 rows/core). Each core quantizes a distinct
512-row slice of w; the quantized bf16 w is AllGathered so each core holds
all of w quantized while doing only 1/8 of the quantization work. amax is
computed on-device: per-core abs-max over its distinct x/w slices, then a
tiny AllReduce(max) collective.
"""

import numpy as np

import concourse.bass as bass
import concourse.bacc as bacc
import concourse.bass_isa as bass_isa
import concourse.mybir as mybir
import concourse.tile as tile
from concourse.bass_utils import run_bass_kernel_spmd

N, K, M = 8192, 4096, 4096
N_CORES = 8
NS = N // N_CORES   # 1024 x rows per core
WS = M // N_CORES   # 512 w rows per core (quantize shard)
P = 128
KB = K // P         # 32 k-blocks
NB = NS // P        # 8 n-blocks per core
MP = 512            # m-panel width
NMP = M // MP       # 8 m-panels
XT = NS // P        # 8 x f32 load tiles
WT = WS // P        # 4 w f32 load tiles

MAGIC = 12582912.0  # 1.5 * 2**23: v + MAGIC - MAGIC == round-half-even(v)
F32 = mybir.dt.float32
BF16 = mybir.dt.bfloat16

_cached_nc = None


def _body(nc, tc, xs, wa, bias_in, out):
    RG = [list(range(N_CORES))]
    xs_t = xs.rearrange("(t p) k -> t p k", p=P)
    wa_t = wa.rearrange("(t p) k -> t p k", p=P)

    with (
        tc.tile_pool(name="const", bufs=1) as const,
        tc.tile_pool(name="dram", bufs=1, space="DRAM") as dram,
    ):
        wq_sh = dram.tile([WS, K], BF16)
        wq_all = dram.tile([M, K], BF16, addr_space="Shared")
        xq = dram.tile([NS, K], BF16)
        cc_in = dram.tile([1, 16], F32)
        cc_out = dram.tile([1, 16], F32, addr_space="Shared")

        # bias broadcast across all 128 partitions: [128, M] f32
        bias_bc = const.tile([P, M], F32)
        bias_b_ap = bass.AP(
            tensor=bias_in.tensor,
            offset=bias_in.offset,
            ap=[[0, P]] + list(bias_in.ap),
        )
        nc.gpsimd.dma_start(out=bias_bc[:], in_=bias_b_ap)

        scl = const.tile([P, 4], F32)  # 0: scale_x, 1: scale_w, 2: dequant, 3: tmp

        with (
            tc.tile_pool(name="ld", bufs=3) as ld,
            tc.tile_pool(name="qs", bufs=2) as qs,
            tc.tile_pool(name="qb", bufs=2) as qbp,
        ):
            # ---- Phase A: local abs-max over this core's distinct slices ----
            partx = const.tile([P, XT], F32)
            partw = const.tile([P, WT], F32)
            for t in range(XT):
                tl = ld.tile([P, K], F32, tag="ldf32", name=f"ldx{t}")
                nc.sync.dma_start(tl[:], xs_t[t])
                nc.vector.tensor_reduce(
                    out=partx[:, t : t + 1], in_=tl[:],
                    op=mybir.AluOpType.max, axis=mybir.AxisListType.X,
                    apply_absolute_value=True,
                )
            for t in range(WT):
                tl = ld.tile([P, K], F32, tag="ldf32", name=f"ldw{t}")
                nc.sync.dma_start(tl[:], wa_t[t])
                nc.vector.tensor_reduce(
                    out=partw[:, t : t + 1], in_=tl[:],
                    op=mybir.AluOpType.max, axis=mybir.AxisListType.X,
                    apply_absolute_value=True,
                )
            px = const.tile([P, 1], F32)
            pw = const.tile([P, 1], F32)
            nc.vector.tensor_reduce(out=px[:], in_=partx[:], op=mybir.AluOpType.max,
                                    axis=mybir.AxisListType.X)
            nc.vector.tensor_reduce(out=pw[:], in_=partw[:], op=mybir.AluOpType.max,
                                    axis=mybir.AxisListType.X)
            rx = const.tile([P, 1], F32)
            rw = const.tile([P, 1], F32)
            nc.gpsimd.partition_all_reduce(rx[:], px[:], channels=P,
                                           reduce_op=bass_isa.ReduceOp.max)
            nc.gpsimd.partition_all_reduce(rw[:], pw[:], channels=P,
                                           reduce_op=bass_isa.ReduceOp.max)

            # ---- AllReduce(max) of [amax_x, amax_w] across the 8 cores ----
            pack = const.tile([1, 16], F32)
            nc.vector.memset(pack[:], 0.0)
            nc.vector.tensor_copy(pack[:1, 0:1], rx[:1, :])
            nc.vector.tensor_copy(pack[:1, 1:2], rw[:1, :])
            nc.sync.dma_start(cc_in[:], pack[:])
            nc.gpsimd.collective_compute(
                "AllReduce", mybir.AluOpType.max,
                ins=[cc_in.opt()], outs=[cc_out.opt()], replica_groups=RG,
            )
            got = const.tile([1, 16], F32)
            nc.sync.dma_start(got[:], cc_out[:])
            gb = const.tile([P, 16], F32)
            nc.gpsimd.partition_broadcast(gb[:], got[:])

            # scale_x = 127/amax_x ; scale_w = 127/amax_w  (reciprocal + mult;
            # <=1-2 ulp off exact division, only shifts exact rounding ties)
            inv = const.tile([P, 2], F32)
            nc.vector.reciprocal(inv[:], gb[:, 0:2])
            nc.vector.tensor_scalar(out=scl[:, 0:2], in0=inv[:], scalar1=127.0,
                                    scalar2=None, op0=mybir.AluOpType.mult)
            # dequant = amax_x * amax_w * (1/16129)
            nc.vector.tensor_tensor(out=scl[:, 3:4], in0=gb[:, 0:1], in1=gb[:, 1:2],
                                    op=mybir.AluOpType.mult)
            nc.vector.tensor_scalar(out=scl[:, 2:3], in0=scl[:, 3:4],
                                    scalar1=float(np.float32(1.0) / np.float32(16129.0)),
                                    scalar2=None, op0=mybir.AluOpType.mult)

            # ---- Phase B: quantize this core's w slice -> bf16, AllGather ----
            wq_t = wq_sh.rearrange("(t p) k -> t p k", p=P)
            for t in range(WT):
                tl = ld.tile([P, K], F32, tag="ldf32", name=f"ldw2{t}")
                nc.sync.dma_start(tl[:], wa_t[t])
                t1 = qs.tile([P, K], F32, tag="t1", name=f"wt1{t}")
                nc.vector.tensor_scalar(out=t1[:], in0=tl[:], scalar1=scl[:, 1:2],
                                        scalar2=MAGIC, op0=mybir.AluOpType.mult,
                                        op1=mybir.AluOpType.add)
                qb = qbp.tile([P, K], BF16, tag="qb", name=f"wqb{t}")
                nc.vector.tensor_scalar(out=qb[:], in0=t1[:], scalar1=MAGIC,
                                        scalar2=None, op0=mybir.AluOpType.subtract)
                nc.sync.dma_start(wq_t[t], qb[:])
            nc.gpsimd.collective_compute(
                "AllGather", mybir.AluOpType.bypass,
                ins=[wq_sh.opt()], outs=[wq_all.opt()], replica_groups=RG,
            )

            # ---- Phase C: quantize this core's x shard -> bf16 in DRAM ----
            xq_t = xq.rearrange("(t p) k -> t p k", p=P)
            for t in range(XT):
                tl = ld.tile([P, K], F32, tag="ldf32", name=f"ldx2{t}")
                nc.sync.dma_start(tl[:], xs_t[t])
                t1 = qs.tile([P, K], F32, tag="t1", name=f"xt1{t}")
                nc.vector.tensor_scalar(out=t1[:], in0=tl[:], scalar1=scl[:, 0:1],
                                        scalar2=MAGIC, op0=mybir.AluOpType.mult,
                                        op1=mybir.AluOpType.add)
                qb = qbp.tile([P, K], BF16, tag="qb", name=f"xqb{t}")
                nc.vector.tensor_scalar(out=qb[:], in0=t1[:], scalar1=MAGIC,
                                        scalar2=None, op0=mybir.AluOpType.subtract)
                nc.sync.dma_start(xq_t[t], qb[:])

        # ---- Phase D: transposed loads + matmuls + fused dequant/bias ----
        with (
            tc.tile_pool(name="xt", bufs=NB) as xtp,
            tc.tile_pool(name="wt", bufs=2) as wtp,
            tc.tile_pool(name="ps", bufs=4, space="PSUM") as psp,
            tc.tile_pool(name="ob", bufs=4) as obp,
        ):
            xT = []
            for nb in range(NB):
                t = xtp.tile([P, KB, P], BF16, tag="xT", name=f"xT{nb}")
                nc.sync.dma_start_transpose(t[:], xq[nb * P : (nb + 1) * P, :])
                xT.append(t)
            for mp in range(NMP):
                wt = wtp.tile([P, KB, MP], BF16, tag="wT", name=f"wT{mp}")
                nc.sync.dma_start_transpose(wt[:], wq_all[mp * MP : (mp + 1) * MP, :])
                for nb in range(NB):
                    ps = psp.tile([P, MP], F32, tag="ps", name=f"ps{mp}_{nb}")
                    for ks in range(KB):
                        nc.tensor.matmul(
                            ps[:], xT[nb][:, ks, :], wt[:, ks, :],
                            start=(ks == 0), stop=(ks == KB - 1),
                        )
                    ob = obp.tile([P, MP], F32, tag="ob", name=f"ob{mp}_{nb}")
                    nc.vector.scalar_tensor_tensor(
                        out=ob[:], in0=ps[:], scalar=scl[:, 2:3],
                        in1=bias_bc[:, mp * MP : (mp + 1) * MP],
                        op0=mybir.AluOpType.mult, op1=mybir.AluOpType.add,
                    )
                    nc.sync.dma_start(
                        out[nb * P : (nb + 1) * P, mp * MP : (mp + 1) * MP], ob[:]
                    )


def _build():
    global _cached_nc
    if _cached_nc is not None:
        return _cached_nc
    nc = bacc.Bacc("TRN2", target_bir_lowering=False, debug=False,
                   num_devices=N_CORES)
    xs = nc.dram_tensor("xs", [NS, K], F32, kind="ExternalInput")
    wa = nc.dram_tensor("wa", [WS, K], F32, kind="ExternalInput")
    bias = nc.dram_tensor("bias", [M], F32, kind="ExternalInput")
    out = nc.dram_tensor("out", [NS, M], F32, kind="ExternalOutput")
    with tile.TileContext(nc) as tc:
        _body(nc, tc, xs.ap(), wa.ap(), bias.ap(), out.ap())
    nc.compile()
    _cached_nc = nc
    return nc


def kernel(x, weight, bias, _trace=False, _trace_kwargs=None):
    x = np.ascontiguousarray(np.asarray(x, dtype=np.float32))
    weight = np.ascontiguousarray(np.asarray(weight, dtype=np.float32))
    bias = np.ascontiguousarray(np.asarray(bias, dtype=np.float32))
    assert x.shape == (N, K) and weight.shape == (M, K) and bias.shape == (M,)

    nc = _build()
    in_maps = [
        {
            "xs": x[c * NS : (c + 1) * NS],
            "wa": weight[c * WS : (c + 1) * WS],
            "bias": bias,
        }
        for c in range(N_CORES)
    ]
    res = run_bass_kernel_spmd(
        nc, in_maps, core_ids=list(range(N_CORES)),
        trace=_trace, **(_trace_kwargs or {}),
    )
    out = np.concatenate([res.results[c]["out"] for c in range(N_CORES)], axis=0)
    if _trace:
        return out, res
    return out
